# revision 1
# baseline (speedup 1.0000x reference)
"""Trainium2 Bass kernel for ConvHex graph message passing.

Computation (per reference):
    out[b,o,h] = (Wc @ x[b,:,h] + sum_k Wn[:,:,k] @ x[b,:,nbr[h,k]]*mask) / counts[h] + bias[o]

Strategy:
  - Data-parallel: shard batch dim B=64 across 8 cores (8 batches/core).
  - Host packs per core:
      xg  [H+1, 512] bf16   gather source; row h = x[b,c,h] b-major flattened,
                            row H is all zeros (target for masked/-1 neighbors).
      xp  [128, 4, HP] bf16 center operand, partition p=64*(b%2)+c, chunk j=b//2.
      idx [128, K, NT, 32] int16 neighbor ids wrapped i -> [i%16, i//16],
                            replicated across the 8 groups of 16 partitions.
      w   [128, 7, 128] bf16 lhsT weights [c, term, o]; rows 64:128 duplicate
                            rows 0:64 so lhsT base_partition can match rhs.
      ic  [128, HP] f32     1/counts replicated across partitions.
      bias[128, 1] f32
  - Device per (k, h-tile of 512): one hardware dma_gather (transpose mode)
    pulls 512 rows of xg into a [128, 4, 512] bf16 SBUF tile whose layout
    matches xp (channels on partitions, gathered h on free dim).
  - Per (h-tile, b): 7 accumulating K=64 matmuls (6 neighbor terms + center)
    into a PSUM bank; DVE multiplies by 1/counts; ACT adds bias; DMA out fp32.
"""

import numpy as np
import ml_dtypes

import concourse.bacc as bacc
import concourse.mybir as mybir
import concourse.tile as tile
from concourse.bass_utils import run_bass_kernel_spmd

BF16 = ml_dtypes.bfloat16

B, C_IN, C_OUT, H, K = 64, 64, 128, 1855, 6
NCORES = 8
BL = B // NCORES            # batches per core
HP = 2048                   # padded H (4 tiles of 512)
NT = HP // 512              # h-tiles
TW = 512                    # h-tile width (= one PSUM bank of f32)
ZROW = H                    # index of the all-zero row in the gather source
ROW = BL * C_IN             # gather row length in elements (512)

_CACHE = {}


def _build_nc():
    nc = bacc.Bacc("TRN2", target_bir_lowering=False, debug=False)

    xg = nc.dram_tensor("xg", [H + 1, ROW], mybir.dt.bfloat16, kind="ExternalInput")
    xp = nc.dram_tensor("xp", [128, BL // 2, HP], mybir.dt.bfloat16, kind="ExternalInput")
    idx = nc.dram_tensor("idx", [128, K, NT, TW // 16], mybir.dt.int16, kind="ExternalInput")
    w = nc.dram_tensor("w", [128, K + 1, C_OUT], mybir.dt.bfloat16, kind="ExternalInput")
    ic = nc.dram_tensor("ic", [128, HP], mybir.dt.float32, kind="ExternalInput")
    bias = nc.dram_tensor("bias", [128, 1], mybir.dt.float32, kind="ExternalInput")
    out = nc.dram_tensor("out", [BL, C_OUT, H], mybir.dt.float32, kind="ExternalOutput")

    with tile.TileContext(nc) as tc:
        with (
            tc.tile_pool(name="const", bufs=1) as const,
            tc.tile_pool(name="gath", bufs=2) as gath,
            tc.tile_pool(name="psum", bufs=4, space="PSUM") as psum,
            tc.tile_pool(name="outp", bufs=4) as outp,
        ):
            idx_sb = const.tile([128, K, NT, TW // 16], mybir.dt.int16)
            nc.sync.dma_start(out=idx_sb[:], in_=idx[:])
            w_sb = const.tile([128, K + 1, C_OUT], mybir.dt.bfloat16)
            nc.sync.dma_start(out=w_sb[:], in_=w[:])
            ic_sb = const.tile([128, HP], mybir.dt.float32)
            nc.sync.dma_start(out=ic_sb[:], in_=ic[:])
            bias_sb = const.tile([128, 1], mybir.dt.float32)
            nc.sync.dma_start(out=bias_sb[:], in_=bias[:])
            xp_sb = const.tile([128, BL // 2, HP], mybir.dt.bfloat16)
            nc.sync.dma_start(out=xp_sb[:], in_=xp[:])

            for t in range(NT):
                g_tiles = []
                for k in range(K):
                    g = gath.tile([128, BL // 2, TW], mybir.dt.bfloat16, tag=f"g{k}")
                    nc.gpsimd.dma_gather(
                        out_ap=g[:],
                        in_ap=xg[:],
                        idxs_ap=idx_sb[:, k, t, :],
                        num_idxs=TW,
                        num_idxs_reg=TW,
                        elem_size=ROW,
                        transpose=True,
                    )
                    g_tiles.append(g)

                for b in range(BL):
                    half = 64 * (b % 2)
                    j = b // 2
                    ps = psum.tile([C_OUT, TW], mybir.dt.float32)
                    for kk in range(K + 1):
                        if kk < K:
                            rhs = g_tiles[kk][half:half + 64, j, :]
                        else:
                            rhs = xp_sb[half:half + 64, j, t * TW:(t + 1) * TW]
                        nc.tensor.matmul(
                            ps[:],
                            w_sb[half:half + 64, kk, :],
                            rhs,
                            start=(kk == 0),
                            stop=(kk == K),
                        )
                    o1 = outp.tile([C_OUT, TW], mybir.dt.float32)
                    nc.vector.tensor_mul(o1[:], ps[:], ic_sb[:, t * TW:(t + 1) * TW])
                    o2 = outp.tile([C_OUT, TW], mybir.dt.float32)
                    nc.scalar.add(o2[:], o1[:], bias_sb[:])
                    wdt = min(H, (t + 1) * TW) - t * TW
                    nc.sync.dma_start(
                        out=out[b, :, t * TW:t * TW + wdt], in_=o2[:, :wdt]
                    )

    nc.compile()
    return nc


def _get_nc():
    if "nc" not in _CACHE:
        _CACHE["nc"] = _build_nc()
    return _CACHE["nc"]


def _prepare_inputs(x, neighbors, weight_center, weight_neighbors, bias):
    """Host-side packing. Returns per-core in_maps."""
    x = np.asarray(x, dtype=np.float32)
    neighbors = np.asarray(neighbors)
    wc = np.asarray(weight_center, dtype=np.float32)
    wn = np.asarray(weight_neighbors, dtype=np.float32)
    bias = np.asarray(bias, dtype=np.float32)

    mask = neighbors >= 0                                     # [H, K]
    idx_hk = np.where(mask, neighbors, ZROW).astype(np.int64)  # [H, K]
    counts = 1.0 + mask.sum(axis=-1).astype(np.float32)        # [H]
    inv_c = (1.0 / counts).astype(np.float32)

    ic_full = np.ones((HP,), np.float32)
    ic_full[:H] = inv_c
    ic_rep = np.ascontiguousarray(np.broadcast_to(ic_full[None, :], (128, HP)))

    # indices: pad to HP with ZROW, wrap i -> [i%16, i//16] per (k, t) tile,
    # replicate across the 8 groups of 16 partitions.
    idx_pad = np.full((K, HP), ZROW, np.int64)
    idx_pad[:, :H] = idx_hk.T
    # [K, NT, TW] -> [K, NT, TW//16, 16] -> [16, K, NT, TW//16]
    wrapped = idx_pad.reshape(K, NT, TW // 16, 16).transpose(3, 0, 1, 2)
    idx_arr = np.ascontiguousarray(
        np.broadcast_to(wrapped[None], (8, 16, K, NT, TW // 16))
    ).reshape(128, K, NT, TW // 16).astype(np.int16)

    # weights lhsT [c, term, o], terms 0..5 = neighbors, 6 = center; dup rows
    wl = np.empty((C_IN, K + 1, C_OUT), np.float32)
    wl[:, :K, :] = wn.transpose(1, 2, 0)
    wl[:, K, :] = wc.T
    w_arr = np.concatenate([wl, wl], axis=0).astype(BF16)      # [128, 7, 128]

    bias_col = np.ascontiguousarray(bias.reshape(C_OUT, 1))

    x_bf = x.astype(BF16)                                      # [B, C, H]

    in_maps = []
    for c in range(NCORES):
        xc = x_bf[c * BL:(c + 1) * BL]                         # [BL, C, H]
        xg_arr = np.zeros((H + 1, ROW), BF16)
        xg_arr[:H] = xc.transpose(2, 0, 1).reshape(H, ROW)     # row h, b-major
        xp_arr = np.zeros((2, C_IN, BL // 2, HP), BF16)
        for b in range(BL):
            xp_arr[b % 2, :, b // 2, :H] = xc[b]
        xp_arr = xp_arr.reshape(128, BL // 2, HP)
        in_maps.append({
            "xg": xg_arr,
            "xp": np.ascontiguousarray(xp_arr),
            "idx": idx_arr,
            "w": w_arr,
            "ic": ic_rep,
            "bias": bias_col,
        })
    return in_maps


def kernel(x, neighbors, weight_center, weight_neighbors, bias):
    nc = _get_nc()
    in_maps = _prepare_inputs(x, neighbors, weight_center, weight_neighbors, bias)
    res = run_bass_kernel_spmd(nc, in_maps, core_ids=list(range(NCORES)))
    out = np.concatenate([r["out"] for r in res.results], axis=0)
    return np.ascontiguousarray(out.astype(np.float32))


# revision 5
# speedup vs baseline: 1.3637x; 1.3637x over previous
"""Trainium2 Bass kernel for ConvHex graph message passing.

Computation (per reference):
    out[b,o,h] = (Wc @ x[b,:,h] + sum_k Wn[:,:,k] @ x[b,:,nbr[h,k]]*mask) / counts[h] + bias[o]

Strategy (v2, H-sharded):
  - Shard the hexagon dim H=1855 (padded to 1856) across 8 cores: 232 h's per
    core, all 64 batches. This makes each gather descriptor move one full
    source column for all batches/channels (64*64 bf16 = 8 KB), which is the
    efficient regime for the SDMA engines (v1's batch-sharding produced 1 KB
    descriptors and ran at half rate).
  - Host packs:
      xg  [1856, 4096] bf16  gather source, row h = x[b,c,h] b-major; row 1855
                             doubles as the all-zero target for masked (-1)
                             neighbors via index remap (h=1855 is H padding and
                             always zero).
      xp  [128, 32, 256] bf16 per-core center operand: partition 64*(b%2)+c,
                             chunk b//2, local h.
      idx [128, 6, 2, 16] int16 per-core neighbor ids per (k, batch-half),
                             wrapped i -> [i%16, i//16], replicated over the
                             8 groups of 16 partitions.
      w   [128, 7, 128] bf16 lhsT weights [c, term, o]; rows 64:128 duplicate
                             rows 0:64 so lhsT base_partition matches rhs.
      ic  [128, 256] f32     1/counts for the core's h-slice, replicated.
      bias[128, 1] f32
  - Device: per (k, batch-half) one hardware dma_gather (transpose mode,
    4 KB descriptors) into [128, 16, 256] bf16 tiles (channels on partitions,
    gathered h on free dim; layout matches xp).
  - Per batch-pair (even b, odd b): 2x7 accumulating K=64 matmuls into two
    PSUM banks. Even-b lhsT/rhs live on partitions 0:64, odd-b on 64:128, so
    the PE runs the two streams on disjoint row-groups concurrently.
  - DVE multiplies by 1/counts (PSUM -> SBUF), ACT adds bias, DMA out f32.
  - Host reassembles the 8 h-slices.
"""

import numpy as np
import ml_dtypes

import concourse.bacc as bacc
import concourse.mybir as mybir
import concourse.tile as tile
from concourse.bass_utils import run_bass_kernel_spmd

BF16 = ml_dtypes.bfloat16

B, C_IN, C_OUT, H, K = 64, 64, 128, 1855, 6
NCORES = 8
HL = 232                    # h's per core (8*232 = 1856 = H+1 pad)
HLP = 256                   # padded to a multiple of 128 for dma_gather
ZROW = H                    # all-zero row in the gather source (h=1855 pad)
ROW = B * C_IN              # gather source row length (4096 elems, 8 KB bf16)
NBH = 2                     # batch halves (gather split for pipelining)

_CACHE = {}


def _build_nc():
    nc = bacc.Bacc("TRN2", target_bir_lowering=False, debug=False)

    xg = nc.dram_tensor("xg", [H + 1, ROW], mybir.dt.bfloat16, kind="ExternalInput")
    xp = nc.dram_tensor("xp", [128, B // 2, HLP], mybir.dt.bfloat16, kind="ExternalInput")
    idx = nc.dram_tensor("idx", [128, K, NBH, HLP // 16], mybir.dt.int16, kind="ExternalInput")
    w = nc.dram_tensor("w", [128, K + 1, C_OUT], mybir.dt.bfloat16, kind="ExternalInput")
    ic = nc.dram_tensor("ic", [128, HLP], mybir.dt.float32, kind="ExternalInput")
    bias = nc.dram_tensor("bias", [128, 1], mybir.dt.float32, kind="ExternalInput")
    out = nc.dram_tensor("out", [B, C_OUT, HL], mybir.dt.float32, kind="ExternalOutput")

    HB = ROW // NBH             # gather elem per batch-half (2048)
    JB = B // 2 // NBH          # batch-pair chunks per half (16)

    with tile.TileContext(nc) as tc:
        with (
            tc.tile_pool(name="const", bufs=1) as const,
            tc.tile_pool(name="gath", bufs=1) as gath,
            tc.tile_pool(name="psum", bufs=4, space="PSUM") as psum,
            tc.tile_pool(name="outp", bufs=4) as outp,
        ):
            idx_sb = const.tile([128, K, NBH, HLP // 16], mybir.dt.int16)
            nc.sync.dma_start(out=idx_sb[:], in_=idx[:])
            w_sb = const.tile([128, K + 1, C_OUT], mybir.dt.bfloat16)
            nc.sync.dma_start(out=w_sb[:], in_=w[:])
            ic_sb = const.tile([128, HLP], mybir.dt.float32)
            nc.sync.dma_start(out=ic_sb[:], in_=ic[:])
            bias_sb = const.tile([128, 1], mybir.dt.float32)
            nc.sync.dma_start(out=bias_sb[:], in_=bias[:])
            xp_sb = const.tile([128, B // 2, HLP], mybir.dt.bfloat16)
            nc.sync.dma_start(out=xp_sb[:], in_=xp[:])

            # gathers: one per (batch-half, neighbor k); 4 KB descriptors
            g_tiles = [[None] * K for _ in range(NBH)]
            for bh in range(NBH):
                for k in range(K):
                    g = gath.tile([128, JB, HLP], mybir.dt.bfloat16,
                                  tag=f"g{bh}_{k}", name=f"g{bh}_{k}")
                    nc.gpsimd.dma_gather(
                        out_ap=g[:],
                        in_ap=xg[:, bh * HB:(bh + 1) * HB],
                        idxs_ap=idx_sb[:, k, bh, :],
                        num_idxs=HLP,
                        num_idxs_reg=HLP,
                        elem_size=HB,
                        elem_step=ROW,
                        transpose=True,
                    )
                    g_tiles[bh][k] = g

            # batch-pair loop: even b on partitions 0:64 (PE rows 0-63),
            # odd b on 64:128 (rows 64-127); interleaved so the PE overlaps
            # the two row-group streams.
            for j in range(B // 2):          # j = b//2
                bh, jl = j // JB, j % JB
                ps = [psum.tile([C_OUT, HLP], mybir.dt.float32,
                                tag=f"ps{par}", name=f"ps{par}_{j}")
                      for par in range(2)]
                for kk in range(K + 1):
                    for par in range(2):     # 0 = even b, 1 = odd b
                        half = 64 * par
                        if kk < K:
                            rhs = g_tiles[bh][kk][half:half + 64, jl, :]
                        else:
                            rhs = xp_sb[half:half + 64, j, :]
                        nc.tensor.matmul(
                            ps[par][:],
                            w_sb[half:half + 64, kk, :],
                            rhs,
                            start=(kk == 0),
                            stop=(kk == K),
                        )
                for par in range(2):
                    o1 = outp.tile([C_OUT, HLP], mybir.dt.float32, tag=f"o1{par}")
                    nc.vector.tensor_mul(o1[:], ps[par][:], ic_sb[:])
                    o2 = outp.tile([C_OUT, HLP], mybir.dt.float32, tag=f"o2{par}")
                    nc.scalar.add(o2[:], o1[:], bias_sb[:])
                    nc.sync.dma_start(out=out[2 * j + par, :, :], in_=o2[:, :HL])

    nc.compile()
    return nc


def _get_nc():
    if "nc" not in _CACHE:
        _CACHE["nc"] = _build_nc()
    return _CACHE["nc"]


def _prepare_inputs(x, neighbors, weight_center, weight_neighbors, bias):
    """Host-side packing. Returns per-core in_maps."""
    x = np.asarray(x, dtype=np.float32)
    neighbors = np.asarray(neighbors)
    wc = np.asarray(weight_center, dtype=np.float32)
    wn = np.asarray(weight_neighbors, dtype=np.float32)
    bias = np.asarray(bias, dtype=np.float32)

    mask = neighbors >= 0                                      # [H, K]
    idx_hk = np.where(mask, neighbors, ZROW).astype(np.int64)  # [H, K]
    counts = 1.0 + mask.sum(axis=-1).astype(np.float32)        # [H]
    inv_c = (1.0 / counts).astype(np.float32)

    x_bf = x.astype(BF16)                                      # [B, C, H]
    # shared gather source: row h = x[:, :, h] flattened b-major; row H = 0
    xg_arr = np.zeros((H + 1, ROW), BF16)
    xg_arr[:H] = x_bf.transpose(2, 0, 1).reshape(H, ROW)

    # weights lhsT [c, term, o], terms 0..5 = neighbors, 6 = center; dup rows
    wl = np.empty((C_IN, K + 1, C_OUT), np.float32)
    wl[:, :K, :] = wn.transpose(1, 2, 0)
    wl[:, K, :] = wc.T
    w_arr = np.concatenate([wl, wl], axis=0).astype(BF16)      # [128, 7, 128]
    bias_col = np.ascontiguousarray(bias.reshape(C_OUT, 1))

    # center operand, all batches: [2, 64, 32, 1856] -> per-core slices
    xp_full = np.zeros((2, C_IN, B // 2, NCORES * HL), BF16)
    for b in range(B):
        xp_full[b % 2, :, b // 2, :H] = x_bf[b]
    xp_full = xp_full.reshape(128, B // 2, NCORES * HL)

    in_maps = []
    for c in range(NCORES):
        h0 = c * HL
        hs = min(HL, H - h0) if h0 < H else 0    # valid h's this core

        ic_arr = np.ones((HLP,), np.float32)
        ic_arr[:hs] = inv_c[h0:h0 + hs]
        ic_rep = np.ascontiguousarray(np.broadcast_to(ic_arr[None, :], (128, HLP)))

        idx_loc = np.full((K, HLP), ZROW, np.int64)
        idx_loc[:, :hs] = idx_hk[h0:h0 + hs].T
        # wrap i -> [i%16, i//16]; replicate over partition groups
        wrapped = idx_loc.reshape(K, HLP // 16, 16).transpose(2, 0, 1)  # [16, K, 16]
        idx_arr = np.ascontiguousarray(
            np.broadcast_to(wrapped[None], (8, 16, K, HLP // 16))
        ).reshape(128, K, HLP // 16).astype(np.int16)
        # same index list for both batch-half gathers
        idx_arr = np.ascontiguousarray(
            np.repeat(idx_arr[:, :, None, :], NBH, axis=2)
        )

        xp_arr = np.zeros((128, B // 2, HLP), BF16)
        xp_arr[:, :, :HL] = xp_full[:, :, h0:h0 + HL]

        in_maps.append({
            "xg": xg_arr,
            "xp": xp_arr,
            "idx": idx_arr,
            "w": w_arr,
            "ic": ic_rep,
            "bias": bias_col,
        })
    return in_maps


def kernel(x, neighbors, weight_center, weight_neighbors, bias):
    nc = _get_nc()
    in_maps = _prepare_inputs(x, neighbors, weight_center, weight_neighbors, bias)
    res = run_bass_kernel_spmd(nc, in_maps, core_ids=list(range(NCORES)))
    out = np.concatenate([r["out"] for r in res.results], axis=2)  # [B, C, 1856]
    return np.ascontiguousarray(out[:, :, :H].astype(np.float32))


# revision 6
# speedup vs baseline: 1.5844x; 1.1619x over previous
"""Trainium2 Bass kernel for ConvHex graph message passing.

Computation (per reference):
    out[b,o,h] = (Wc @ x[b,:,h] + sum_k Wn[:,:,k] @ x[b,:,nbr[h,k]]*mask) / counts[h] + bias[o]

Strategy (v3, H-sharded):
  - Shard the hexagon dim H=1855 (padded to 1856) across 8 cores: 232 h's per
    core, all 64 batches. Each gather descriptor then moves one source column
    for 32 batches (4 KB) — the efficient SDMA regime.
  - Host packs:
      xg  [1856, 4096] bf16  gather source, row h = x[b,c,h] b-major; row 1855
                             doubles as the all-zero target for masked (-1)
                             neighbors via index remap (h=1855 is H padding and
                             always zero).
      xp  [128, 32, 256] bf16 per-core center operand: partition 64*(b%2)+c,
                             chunk b//2, local h.
      idx [128, 6, 2, 16] int16 per-core neighbor ids per (k, batch-half),
                             wrapped i -> [i%16, i//16], replicated over the
                             8 groups of 16 partitions.
      w   [128, 7, 128] bf16 lhsT weights [c, term, o]; rows 64:128 duplicate
                             rows 0:64 so lhsT base_partition matches rhs.
      ic  [128, 256] f32     1/counts for the core's h-slice, replicated.
      bias[128, 1] f32
  - Device: per (batch-half, k) one hardware dma_gather (transpose mode) into
    [128, 16, 256] bf16 tiles (channels on partitions, gathered h on free dim,
    matching xp's layout). Gathers alternate between 2 SWDGE queues.
  - Per batch-pair j: 2x7 accumulating K=64 matmuls (center term first) into
    two PSUM banks; even b on PE rows 0:64, odd b on rows 64:128, interleaved
    so the row-groups run concurrently.
  - DVE multiplies by 1/counts (PSUM -> SBUF), ACT adds bias writing into an
    8-batch staging tile, one DMA store per 8 batches (7.4 KB descriptors).
  - Host reassembles the 8 h-slices.
"""

import numpy as np
import ml_dtypes

import concourse.bacc as bacc
import concourse.mybir as mybir
import concourse.tile as tile
from concourse.bass_utils import run_bass_kernel_spmd

BF16 = ml_dtypes.bfloat16

B, C_IN, C_OUT, H, K = 64, 64, 128, 1855, 6
NCORES = 8
HL = 232                    # h's per core (8*232 = 1856 = H+1 pad)
HLP = 256                   # gather-padded h count (multiple of 128)
ZROW = H                    # all-zero row in the gather source (h=1855 pad)
ROW = B * C_IN              # gather source row length (4096 elems, 8 KB bf16)
NBH = 2                     # batch halves (gather split for pipelining)
SG = 8                      # batches per store group

_CACHE = {}


def _build_nc():
    nc = bacc.Bacc(
        "TRN2",
        target_bir_lowering=False,
        debug=False,
        num_swdge_queues=2,
        dynamic_dma_scratch_size=32768,
    )

    xg = nc.dram_tensor("xg", [H + 1, ROW], mybir.dt.bfloat16, kind="ExternalInput")
    xp = nc.dram_tensor("xp", [128, B // 2, HLP], mybir.dt.bfloat16, kind="ExternalInput")
    idx = nc.dram_tensor("idx", [128, K, NBH, HLP // 16], mybir.dt.int16, kind="ExternalInput")
    w = nc.dram_tensor("w", [128, K + 1, C_OUT], mybir.dt.bfloat16, kind="ExternalInput")
    ic = nc.dram_tensor("ic", [128, HLP], mybir.dt.float32, kind="ExternalInput")
    bias = nc.dram_tensor("bias", [128, 1], mybir.dt.float32, kind="ExternalInput")
    out = nc.dram_tensor("out", [B // SG, C_OUT, SG, HL], mybir.dt.float32,
                         kind="ExternalOutput")

    HB = ROW // NBH             # gather elem per batch-half (2048)
    JB = B // 2 // NBH          # batch-pair chunks per half (16)
    # accumulation order: center first (its operand loads early), then k 0..5
    terms = [K] + list(range(K))

    with tile.TileContext(nc) as tc:
        with (
            tc.tile_pool(name="const", bufs=1) as const,
            tc.tile_pool(name="gath", bufs=1) as gath,
            tc.tile_pool(name="psum", bufs=4, space="PSUM") as psum,
            tc.tile_pool(name="outp", bufs=4) as outp,
            tc.tile_pool(name="stg", bufs=2) as stg,
        ):
            idx_sb = const.tile([128, K, NBH, HLP // 16], mybir.dt.int16)
            nc.sync.dma_start(out=idx_sb[:], in_=idx[:])
            w_sb = const.tile([128, K + 1, C_OUT], mybir.dt.bfloat16)
            nc.sync.dma_start(out=w_sb[:], in_=w[:])
            ic_sb = const.tile([128, HLP], mybir.dt.float32)
            nc.sync.dma_start(out=ic_sb[:], in_=ic[:])
            bias_sb = const.tile([128, 1], mybir.dt.float32)
            nc.sync.dma_start(out=bias_sb[:], in_=bias[:])
            xp_sb = const.tile([128, B // 2, HLP], mybir.dt.bfloat16)
            nc.sync.dma_start(out=xp_sb[:], in_=xp[:])

            # gathers: one per (batch-half, neighbor k); 4 KB descriptors
            g_tiles = [[None] * K for _ in range(NBH)]
            for bh in range(NBH):
                for k in range(K):
                    g = gath.tile([128, JB, HLP], mybir.dt.bfloat16,
                                  tag=f"g{bh}_{k}", name=f"g{bh}_{k}")
                    nc.gpsimd.dma_gather(
                        out_ap=g[:],
                        in_ap=xg[:, bh * HB:(bh + 1) * HB],
                        idxs_ap=idx_sb[:, k, bh, :],
                        num_idxs=HLP,
                        num_idxs_reg=HLP,
                        elem_size=HB,
                        elem_step=ROW,
                        transpose=True,
                        queue_num=(bh * K + k) % 2,
                    )
                    g_tiles[bh][k] = g

            # batch-pair loop: even b on partitions 0:64 (PE rows 0-63),
            # odd b on 64:128 (rows 64-127), interleaved for row-group overlap
            stage = None
            for j in range(B // 2):          # j = b//2
                bh, jl = j // JB, j % JB
                if j % (SG // 2) == 0:
                    stage = stg.tile([128, SG, HL], mybir.dt.float32,
                                     tag="stage", name=f"stage_{j}")
                ps = [psum.tile([C_OUT, HL], mybir.dt.float32,
                                tag=f"ps{par}", name=f"ps{par}_{j}")
                      for par in range(2)]
                for i, kk in enumerate(terms):
                    for par in range(2):     # 0 = even b, 1 = odd b
                        half = 64 * par
                        if kk < K:
                            rhs = g_tiles[bh][kk][half:half + 64, jl, :HL]
                        else:
                            rhs = xp_sb[half:half + 64, j, :HL]
                        nc.tensor.matmul(
                            ps[par][:],
                            w_sb[half:half + 64, kk, :],
                            rhs,
                            start=(i == 0),
                            stop=(i == K),
                        )
                for par in range(2):
                    o1 = outp.tile([C_OUT, HL], mybir.dt.float32,
                                   tag=f"o1{par}", name=f"o1{par}_{j}")
                    nc.vector.tensor_mul(o1[:], ps[par][:], ic_sb[:, :HL])
                    s = 2 * (j % (SG // 2)) + par
                    nc.scalar.add(stage[:, s, :], o1[:], bias_sb[:])
                if j % (SG // 2) == SG // 2 - 1:
                    nc.sync.dma_start(out=out[j // (SG // 2), :, :, :], in_=stage[:])

    nc.compile()
    return nc


def _get_nc():
    if "nc" not in _CACHE:
        _CACHE["nc"] = _build_nc()
    return _CACHE["nc"]


def _assemble_core(arr):
    """[B//SG, C_OUT, SG, HL] -> [B, C_OUT, HL]"""
    return arr.transpose(0, 2, 1, 3).reshape(B, C_OUT, HL)


def _prepare_inputs(x, neighbors, weight_center, weight_neighbors, bias):
    """Host-side packing. Returns per-core in_maps."""
    x = np.asarray(x, dtype=np.float32)
    neighbors = np.asarray(neighbors)
    wc = np.asarray(weight_center, dtype=np.float32)
    wn = np.asarray(weight_neighbors, dtype=np.float32)
    bias = np.asarray(bias, dtype=np.float32)

    mask = neighbors >= 0                                      # [H, K]
    idx_hk = np.where(mask, neighbors, ZROW).astype(np.int64)  # [H, K]
    counts = 1.0 + mask.sum(axis=-1).astype(np.float32)        # [H]
    inv_c = (1.0 / counts).astype(np.float32)

    x_bf = x.astype(BF16)                                      # [B, C, H]
    # shared gather source: row h = x[:, :, h] flattened b-major; row H = 0
    xg_arr = np.zeros((H + 1, ROW), BF16)
    xg_arr[:H] = x_bf.transpose(2, 0, 1).reshape(H, ROW)

    # weights lhsT [c, term, o], terms 0..5 = neighbors, 6 = center; dup rows
    wl = np.empty((C_IN, K + 1, C_OUT), np.float32)
    wl[:, :K, :] = wn.transpose(1, 2, 0)
    wl[:, K, :] = wc.T
    w_arr = np.concatenate([wl, wl], axis=0).astype(BF16)      # [128, 7, 128]
    bias_col = np.ascontiguousarray(bias.reshape(C_OUT, 1))

    # center operand, all batches: [2, 64, 32, 1856] -> per-core slices
    xp_full = np.zeros((2, C_IN, B // 2, NCORES * HL), BF16)
    for b in range(B):
        xp_full[b % 2, :, b // 2, :H] = x_bf[b]
    xp_full = xp_full.reshape(128, B // 2, NCORES * HL)

    in_maps = []
    for c in range(NCORES):
        h0 = c * HL
        hs = min(HL, H - h0) if h0 < H else 0    # valid h's this core

        ic_arr = np.ones((HLP,), np.float32)
        ic_arr[:hs] = inv_c[h0:h0 + hs]
        ic_rep = np.ascontiguousarray(np.broadcast_to(ic_arr[None, :], (128, HLP)))

        idx_loc = np.full((K, HLP), ZROW, np.int64)
        idx_loc[:, :hs] = idx_hk[h0:h0 + hs].T
        # wrap i -> [i%16, i//16]; replicate over partition groups
        wrapped = idx_loc.reshape(K, HLP // 16, 16).transpose(2, 0, 1)  # [16, K, 16]
        idx_arr = np.ascontiguousarray(
            np.broadcast_to(wrapped[None], (8, 16, K, HLP // 16))
        ).reshape(128, K, HLP // 16).astype(np.int16)
        # same index list for both batch-half gathers
        idx_arr = np.ascontiguousarray(
            np.repeat(idx_arr[:, :, None, :], NBH, axis=2)
        )

        xp_arr = np.zeros((128, B // 2, HLP), BF16)
        xp_arr[:, :, :HL] = xp_full[:, :, h0:h0 + HL]

        in_maps.append({
            "xg": xg_arr,
            "xp": xp_arr,
            "idx": idx_arr,
            "w": w_arr,
            "ic": ic_rep,
            "bias": bias_col,
        })
    return in_maps


def kernel(x, neighbors, weight_center, weight_neighbors, bias):
    nc = _get_nc()
    in_maps = _prepare_inputs(x, neighbors, weight_center, weight_neighbors, bias)
    res = run_bass_kernel_spmd(nc, in_maps, core_ids=list(range(NCORES)))
    out = np.concatenate(
        [_assemble_core(r["out"]) for r in res.results], axis=2)  # [B, C, 1856]
    return np.ascontiguousarray(out[:, :, :H].astype(np.float32))


# revision 13
# speedup vs baseline: 1.6728x; 1.0558x over previous
"""Trainium2 Bass kernel for ConvHex graph message passing.

Computation (per reference):
    out[b,o,h] = (Wc @ x[b,:,h] + sum_k Wn[:,:,k] @ x[b,:,nbr[h,k]]*mask) / counts[h] + bias[o]

Strategy (v3, H-sharded):
  - Shard the hexagon dim H=1855 (padded to 1856) across 8 cores: 232 h's per
    core, all 64 batches. Each gather descriptor then moves one source column
    for 32 batches (4 KB) — the efficient SDMA regime.
  - Host packs:
      xg  [1856, 4096] bf16  gather source, row h = x[b,c,h] b-major; row 1855
                             doubles as the all-zero target for masked (-1)
                             neighbors via index remap (h=1855 is H padding and
                             always zero).
      xp  [128, 32, 256] bf16 per-core center operand: partition 64*(b%2)+c,
                             chunk b//2, local h.
      idx [128, 6, 2, 16] int16 per-core neighbor ids per (k, batch-half),
                             wrapped i -> [i%16, i//16], replicated over the
                             8 groups of 16 partitions.
      w   [128, 7, 128] bf16 lhsT weights [c, term, o]; rows 64:128 duplicate
                             rows 0:64 so lhsT base_partition matches rhs.
      ic  [128, 256] f32     1/counts for the core's h-slice, replicated.
      bias[128, 1] f32
  - Device: per (batch-half, k) one hardware dma_gather (transpose mode) into
    [128, 16, 256] bf16 tiles (channels on partitions, gathered h on free dim,
    matching xp's layout). Gathers alternate between 2 SWDGE queues.
  - Per batch-pair j: 2x7 accumulating K=64 matmuls (center term first) into
    two PSUM banks; even b on PE rows 0:64, odd b on rows 64:128, interleaved
    so the row-groups run concurrently.
  - DVE multiplies by 1/counts (PSUM -> SBUF), ACT adds bias writing into an
    8-batch staging tile, one DMA store per 8 batches (7.4 KB descriptors).
  - Host reassembles the 8 h-slices.
"""

import numpy as np
import ml_dtypes

import concourse.bacc as bacc
import concourse.mybir as mybir
import concourse.tile as tile
from concourse.bass_utils import run_bass_kernel_spmd

BF16 = ml_dtypes.bfloat16

B, C_IN, C_OUT, H, K = 64, 64, 128, 1855, 6
NCORES = 8
HL = 232                    # h's per core (8*232 = 1856 = H+1 pad)
HLP = 256                   # gather-padded h count (multiple of 128)
ZROW = H                    # all-zero row in the gather source (h=1855 pad)
ROW = B * C_IN              # gather source row length (4096 elems, 8 KB bf16)
NBH = 2                     # batch halves (gather split for pipelining)
SG = 8                      # batches per store group

_CACHE = {}


def _build_nc():
    nc = bacc.Bacc(
        "TRN2",
        target_bir_lowering=False,
        debug=False,
        num_swdge_queues=2,
        dynamic_dma_scratch_size=32768,
    )

    xg = nc.dram_tensor("xg", [H + 1, ROW], mybir.dt.bfloat16, kind="ExternalInput")
    xp = nc.dram_tensor("xp", [128, B // 2, HLP], mybir.dt.bfloat16, kind="ExternalInput")
    idx = nc.dram_tensor("idx", [128, K, NBH, HLP // 16], mybir.dt.int16, kind="ExternalInput")
    w = nc.dram_tensor("w", [128, K + 1, C_OUT], mybir.dt.bfloat16, kind="ExternalInput")
    ic = nc.dram_tensor("ic", [128, HLP], mybir.dt.float32, kind="ExternalInput")
    # bias folded into the matmul: psum += bias[o] * counts[h], then *1/counts
    biasr = nc.dram_tensor("biasr", [1, C_OUT], mybir.dt.bfloat16, kind="ExternalInput")
    cnt = nc.dram_tensor("cnt", [1, HLP], mybir.dt.bfloat16, kind="ExternalInput")
    out = nc.dram_tensor("out", [B // SG, C_OUT, SG, HL], mybir.dt.float32,
                         kind="ExternalOutput")

    HB = ROW // NBH             # gather elem per batch-half (2048)
    JB = B // 2 // NBH          # batch-pair chunks per half (16)
    # accumulation order: center first (its operand loads early), then k 0..5
    terms = [K] + list(range(K))

    with tile.TileContext(nc) as tc:
        with (
            tc.tile_pool(name="const", bufs=1) as const,
            tc.tile_pool(name="gath", bufs=1) as gath,
            tc.tile_pool(name="psum", bufs=4, space="PSUM") as psum,
            tc.tile_pool(name="stg", bufs=4) as stg,
        ):
            idx_sb = const.tile([128, K, NBH, HLP // 16], mybir.dt.int16)
            nc.sync.dma_start(out=idx_sb[:], in_=idx[:])
            w_sb = const.tile([128, K + 1, C_OUT], mybir.dt.bfloat16)
            nc.sync.dma_start(out=w_sb[:], in_=w[:])
            ic_sb = const.tile([128, HLP], mybir.dt.float32)
            nc.sync.dma_start(out=ic_sb[:], in_=ic[:])
            biasr_sb = const.tile([1, C_OUT], mybir.dt.bfloat16)
            nc.sync.dma_start(out=biasr_sb[:], in_=biasr[:])
            cnt_sb = const.tile([1, HLP], mybir.dt.bfloat16)
            nc.sync.dma_start(out=cnt_sb[:], in_=cnt[:])
            xp_sb = const.tile([128, B // 2, HLP], mybir.dt.bfloat16)
            nc.sync.dma_start(out=xp_sb[:], in_=xp[:])

            # gathers: one per (batch-half, neighbor k); 4 KB descriptors
            g_tiles = [[None] * K for _ in range(NBH)]
            for bh in range(NBH):
                for k in range(K):
                    g = gath.tile([128, JB, HLP], mybir.dt.bfloat16,
                                  tag=f"g{bh}_{k}", name=f"g{bh}_{k}")
                    nc.gpsimd.dma_gather(
                        out_ap=g[:],
                        in_ap=xg[:, bh * HB:(bh + 1) * HB],
                        idxs_ap=idx_sb[:, k, bh, :],
                        num_idxs=HLP,
                        num_idxs_reg=HLP,
                        elem_size=HB,
                        elem_step=ROW,
                        transpose=True,
                        queue_num=(bh * K + k) % 2,
                    )
                    g_tiles[bh][k] = g

            # batch-pair loop: even b on partitions 0:64 (PE rows 0-63),
            # odd b on 64:128 (rows 64-127), interleaved for row-group overlap
            stage = None
            for j in range(B // 2):          # j = b//2
                bh, jl = j // JB, j % JB
                if j % (SG // 2) == 0:
                    stage = stg.tile([128, SG, HL], mybir.dt.float32,
                                     tag="stage", name=f"stage_{j}")
                ps = [psum.tile([C_OUT, HL], mybir.dt.float32,
                                tag=f"ps{par}", name=f"ps{par}_{j}")
                      for par in range(2)]
                for i, kk in enumerate(terms):
                    for par in range(2):     # 0 = even b, 1 = odd b
                        half = 64 * par
                        if kk < K:
                            rhs = g_tiles[bh][kk][half:half + 64, jl, :HL]
                        else:
                            rhs = xp_sb[half:half + 64, j, :HL]
                        nc.tensor.matmul(
                            ps[par][:],
                            w_sb[half:half + 64, kk, :],
                            rhs,
                            start=(i == 0),
                            stop=False,
                        )
                for par in range(2):
                    # rank-1 bias*counts term closes the accumulation group
                    nc.tensor.matmul(
                        ps[par][:], biasr_sb[:], cnt_sb[:, :HL],
                        start=False, stop=True,
                    )
                for par in range(2):
                    s = 2 * (j % (SG // 2)) + par
                    nc.vector.tensor_mul(stage[:, s, :], ps[par][:], ic_sb[:, :HL])
                if j % (SG // 2) == SG // 2 - 1:
                    nc.sync.dma_start(out=out[j // (SG // 2), :, :, :], in_=stage[:])

    nc.compile()
    return nc


def _get_nc():
    if "nc" not in _CACHE:
        _CACHE["nc"] = _build_nc()
    return _CACHE["nc"]


def _assemble_core(arr):
    """[B//SG, C_OUT, SG, HL] -> [B, C_OUT, HL]"""
    return arr.transpose(0, 2, 1, 3).reshape(B, C_OUT, HL)


def _prepare_inputs(x, neighbors, weight_center, weight_neighbors, bias):
    """Host-side packing. Returns per-core in_maps."""
    x = np.asarray(x, dtype=np.float32)
    neighbors = np.asarray(neighbors)
    wc = np.asarray(weight_center, dtype=np.float32)
    wn = np.asarray(weight_neighbors, dtype=np.float32)
    bias = np.asarray(bias, dtype=np.float32)

    mask = neighbors >= 0                                      # [H, K]
    idx_hk = np.where(mask, neighbors, ZROW).astype(np.int64)  # [H, K]
    counts = 1.0 + mask.sum(axis=-1).astype(np.float32)        # [H]
    inv_c = (1.0 / counts).astype(np.float32)

    x_bf = x.astype(BF16)                                      # [B, C, H]
    # shared gather source: row h = x[:, :, h] flattened b-major; row H = 0
    xg_arr = np.zeros((H + 1, ROW), BF16)
    xg_arr[:H] = x_bf.transpose(2, 0, 1).reshape(H, ROW)

    # weights lhsT [c, term, o], terms 0..5 = neighbors, 6 = center; dup rows
    wl = np.empty((C_IN, K + 1, C_OUT), np.float32)
    wl[:, :K, :] = wn.transpose(1, 2, 0)
    wl[:, K, :] = wc.T
    w_arr = np.concatenate([wl, wl], axis=0).astype(BF16)      # [128, 7, 128]
    biasr_arr = np.ascontiguousarray(bias.reshape(1, C_OUT)).astype(BF16)

    # center operand, all batches: [2, 64, 32, 1856] -> per-core slices
    xp_full = np.zeros((2, C_IN, B // 2, NCORES * HL), BF16)
    for b in range(B):
        xp_full[b % 2, :, b // 2, :H] = x_bf[b]
    xp_full = xp_full.reshape(128, B // 2, NCORES * HL)

    in_maps = []
    for c in range(NCORES):
        h0 = c * HL
        hs = min(HL, H - h0) if h0 < H else 0    # valid h's this core

        ic_arr = np.ones((HLP,), np.float32)
        ic_arr[:hs] = inv_c[h0:h0 + hs]
        ic_rep = np.ascontiguousarray(np.broadcast_to(ic_arr[None, :], (128, HLP)))
        cnt_arr = np.zeros((1, HLP), BF16)
        cnt_arr[0, :hs] = counts[h0:h0 + hs].astype(BF16)

        idx_loc = np.full((K, HLP), ZROW, np.int64)
        idx_loc[:, :hs] = idx_hk[h0:h0 + hs].T
        # wrap i -> [i%16, i//16]; replicate over partition groups
        wrapped = idx_loc.reshape(K, HLP // 16, 16).transpose(2, 0, 1)  # [16, K, 16]
        idx_arr = np.ascontiguousarray(
            np.broadcast_to(wrapped[None], (8, 16, K, HLP // 16))
        ).reshape(128, K, HLP // 16).astype(np.int16)
        # same index list for both batch-half gathers
        idx_arr = np.ascontiguousarray(
            np.repeat(idx_arr[:, :, None, :], NBH, axis=2)
        )

        xp_arr = np.zeros((128, B // 2, HLP), BF16)
        xp_arr[:, :, :HL] = xp_full[:, :, h0:h0 + HL]

        in_maps.append({
            "xg": xg_arr,
            "xp": xp_arr,
            "idx": idx_arr,
            "w": w_arr,
            "ic": ic_rep,
            "biasr": biasr_arr,
            "cnt": cnt_arr,
        })
    return in_maps


def kernel(x, neighbors, weight_center, weight_neighbors, bias):
    nc = _get_nc()
    in_maps = _prepare_inputs(x, neighbors, weight_center, weight_neighbors, bias)
    res = run_bass_kernel_spmd(nc, in_maps, core_ids=list(range(NCORES)))
    out = np.concatenate(
        [_assemble_core(r["out"]) for r in res.results], axis=2)  # [B, C, 1856]
    return np.ascontiguousarray(out[:, :, :H].astype(np.float32))
